# revision 23
# baseline (speedup 1.0000x reference)
"""Trainium2 Bass kernel for nn_Diffuse: 1x1conv+BN -> affinity softmax P
-> diffusion step, SPMD over 8 NeuronCores.

Sharding: data-parallel over batch (4) x sequence-parallel over the hw
token dim (2 row-blocks of 2048), one (batch, row-block) per core. Row-
block selection is done by rotating the token axis on the host (softmax
is permutation-equivariant), so all cores run the same program.

Per core (shapes hardcoded for b=4, c=21, h=w=64, cin=512, co=64):
  phase 0: Fv[64,4096] = W'·feat + b'  (BN folded on host, split-bf16 mms)
  phase A: for 16 i-tiles [128 rows]: W-tile = Fv_i^T Fv via 4-term
           split-bf16 mms (exact to ~2^-17), exp on ScalarE with row-sum
           accum, normalize on VectorE, DMA P tile out.  Softmax without
           max-subtraction: logits are bounded (~25), exp stays in fp32.
  phase B: recompute W^T tiles [j=128, i=1024] (2-term split), exp->bf16,
           accumulate out[c,i] += ptk[j,c]^T · E^T[j,i] on PE, normalize
           by 1/rowsum (transposed via PE + broadcast via DRAM bounce).
Host folds BN into conv weights, alpha/beta sigmoids into scalars, and
applies the final seed/pred affine mix after gathering.
"""

import numpy as np
import ml_dtypes

import concourse.bass as bass
from concourse import mybir
from concourse.bass_utils import run_bass_kernel_spmd

BF16 = ml_dtypes.bfloat16
F32 = mybir.dt.float32
BF = mybir.dt.bfloat16

B, C, H, W = 4, 21, 64, 64
HW = H * W                     # 4096
CIN, CO = 512, 64
NCORES = 8
RB = HW * B // NCORES          # 2048 rows per core
IT = RB // 128                 # 16 i-tiles
NS = RB // 512                 # 4 supertiles
NP = RB // 1024                # 2 supertile-pairs (phase B)
JC = HW // 128                 # 32 j-chunks
BN_EPS = 1e-5

_CACHED_NC = None


def build_kernel():
    nc = bass.Bass("TRN2", target_bir_lowering=False, debug=False)

    # ---- DRAM I/O ----
    d_fs = nc.dram_tensor("fs", [128, HW], BF, kind="ExternalInput").ap()
    d_fr = nc.dram_tensor("fr", [128, HW], BF, kind="ExternalInput").ap()
    d_fhh = nc.dram_tensor("fhh", [128, HW], BF, kind="ExternalInput").ap()
    d_ptk = nc.dram_tensor("ptk", [HW, C], BF, kind="ExternalInput").ap()
    d_P = nc.dram_tensor("P_blk", [RB, HW], F32, kind="ExternalOutput").ap()
    d_rs = nc.dram_tensor("rs_blk", [128, IT], F32, kind="ExternalOutput").ap()
    d_out = nc.dram_tensor("out_blk", [C, RB], F32, kind="ExternalOutput").ap()

    # ---- SBUF ----
    sb_ptk = nc.alloc_sbuf_tensor("sb_ptk", [128, JC, C], BF).ap()
    Fs = nc.alloc_sbuf_tensor("Fs", [128, HW], BF).ap()    # [hi;lo]
    Fr = nc.alloc_sbuf_tensor("Fr", [128, HW], BF).ap()    # [lo;hi]
    Fhh = nc.alloc_sbuf_tensor("Fhh", [128, HW], BF).ap()  # [hi;hi]
    Et = nc.alloc_sbuf_tensor("Et", [128, 2, HW], F32).ap()
    Pt = nc.alloc_sbuf_tensor("Pt", [128, 3, HW], F32).ap()
    sp = nc.alloc_sbuf_tensor("sp", [128, IT, 4], F32).ap()
    ssum = nc.alloc_sbuf_tensor("ssum", [128, IT, 1], F32).ap()
    srec = nc.alloc_sbuf_tensor("srec", [128, IT, 1], F32).ap()
    out_raw = nc.alloc_sbuf_tensor("out_raw", [C, RB], F32).ap()
    warm = nc.alloc_sbuf_tensor("warm", [1, 1], F32).ap()
    ETb = nc.alloc_sbuf_tensor("ETb", [128, 3, 1024], BF).ap()

    def PT(t):
        return Pt[:, t % 3, :]

    # ---- PSUM: psA 2x[128,1024] (4 banks, phase-A matmuls; psA[0] also
    #      hosts the rs transpose at the end), psB 1x[128,1024] (2 banks,
    #      W^T chunks), psO 2x[21,512] (2 banks, out accumulators) ----
    psA = [nc.alloc_psum_tensor(f"psA{u}", [128, 1024], F32).ap() for u in range(2)]
    psB = nc.alloc_psum_tensor("psB", [128, 1024], F32).ap()
    psO = [nc.alloc_psum_tensor(f"psO{u}", [C, 512], F32).ap() for u in range(2)]

    # ---- semaphores ----
    sINF = nc.alloc_semaphore("sINF")   # fs, fr first halves
    sINF2 = nc.alloc_semaphore("sINF2")  # fs, fr second halves
    sINH = nc.alloc_semaphore("sINH")   # fhh
    sINP = nc.alloc_semaphore("sINP")   # ptk
    sDVE = nc.alloc_semaphore("sDVE")
    sWMA = nc.alloc_semaphore("sWMA")
    sEXA = nc.alloc_semaphore("sEXA")
    sNRM = nc.alloc_semaphore("sNRM")
    sPOUTs = [nc.alloc_semaphore(f"sPOUT{t}") for t in range(IT)]
    sFIN = nc.alloc_semaphore("sFIN")
    sWARM = nc.alloc_semaphore("sWARM")
    sWMB = nc.alloc_semaphore("sWMB")
    sEXB = nc.alloc_semaphore("sEXB")
    sOMM = nc.alloc_semaphore("sOMM")
    sOCP = nc.alloc_semaphore("sOCP")

    ID = mybir.ActivationFunctionType.Identity
    EXP = mybir.ActivationFunctionType.Exp

    with nc.Block() as block:

        @block.sync
        def _(sp_eng: bass.BassEngine):
            sp_eng.dma_start(out=Fs[:, 0:2048], in_=d_fs[:, 0:2048]
                             ).then_inc(sINF, 16)
            sp_eng.dma_start(out=Fr[:, 0:2048], in_=d_fr[:, 0:2048]
                             ).then_inc(sINF, 16)
            sp_eng.dma_start(out=Fs[:, 2048:4096], in_=d_fs[:, 2048:4096]
                             ).then_inc(sINF2, 16)
            sp_eng.dma_start(out=Fr[:, 2048:4096], in_=d_fr[:, 2048:4096]
                             ).then_inc(sINF2, 16)
            sp_eng.dma_start(out=Fhh, in_=d_fhh).then_inc(sINH, 16)
            sp_eng.dma_start(
                out=sb_ptk, in_=d_ptk.rearrange("(j p) c -> p j c", p=128)
            ).then_inc(sINP, 16)

            # phase A: P tile writes
            for t in range(IT):
                sp_eng.wait_ge(sNRM, t + 1)
                sp_eng.dma_start(
                    out=d_P[128 * t : 128 * (t + 1), :], in_=PT(t)
                ).then_inc(sPOUTs[t], 16)

            # rs out: [128,16] reciprocal row-sums (host transposes)
            sp_eng.wait_ge(sDVE, 2 * IT)
            sp_eng.dma_start(out=d_rs, in_=srec[:, :, 0]).then_inc(sFIN, 16)

            # final out (raw, host multiplies by rs)
            sp_eng.wait_ge(sOCP, 4)
            sp_eng.dma_start(out=d_out, in_=out_raw).then_inc(sFIN, 16)

        @block.gpsimd
        def _(gp: bass.BassEngine):
            gp.memset(warm, 0.0).then_inc(sWARM)

        @block.tensor
        def _(pe: bass.BassEngine):
            # ---- fused rounds r=0..63: phase-A unit r + phase-B step r ----
            pe.wait_ge(sINF, 32)
            pe.wait_ge(sINH, 16)
            pe.wait_ge(sINP, 16)
            inf2_waited = [False]

            def mm_out(r2):
                p2, jc2 = r2 // JC, r2 % JC
                if jc2 == 0 and p2 >= 1:
                    pe.wait_ge(sOCP, 2 * p2)
                pe.wait_ge(sEXB, r2 + 1)
                pe.matmul(psO[0], sb_ptk[:, jc2, :],
                          ETb[:, r2 % 3, 0:512],
                          start=(jc2 == 0), stop=(jc2 == JC - 1))
                pe.matmul(psO[1], sb_ptk[:, jc2, :],
                          ETb[:, r2 % 3, 512:1024],
                          start=(jc2 == 0), stop=(jc2 == JC - 1)
                          ).then_inc(sOMM)

            for r in range(4 * IT):
                t, g = r // 4, r % 4
                u = r % 2
                # phase-A unit r -> psA[u]
                fs_i = Fs[:, bass.ts(t, 128)]
                if g >= 2 and not inf2_waited[0]:
                    pe.wait_ge(sINF2, 32)
                    inf2_waited[0] = True
                if r >= 2:
                    pe.wait_ge(sEXA, r - 1)
                for h in range(2):
                    jr = 1024 * g + 512 * h
                    dst = psA[u][:, bass.ds(512 * h, 512)]
                    pe.matmul(dst, fs_i, Fs[:, bass.ds(jr, 512)],
                              start=True, stop=False)
                    mm = pe.matmul(dst, fs_i, Fr[:, bass.ds(jr, 512)],
                                   start=False, stop=True)
                mm.then_inc(sWMA)
                # phase-B W^T chunk r -> psB (single buffer)
                pb_, jc = r // JC, r % JC
                if r >= 1:
                    pe.wait_ge(sEXB, r)
                fs_j = Fs[:, bass.ts(jc, 128)]
                pe.matmul(psB[:, 0:512], fs_j,
                          Fhh[:, bass.ds(1024 * pb_, 512)],
                          start=True, stop=True)
                pe.matmul(psB[:, 512:1024], fs_j,
                          Fhh[:, bass.ds(1024 * pb_ + 512, 512)],
                          start=True, stop=True).then_inc(sWMB)
                # phase-B out matmuls for round r-1
                if r >= 1:
                    mm_out(r - 1)
            mm_out(4 * IT - 1)

        @block.scalar
        def _(act: bass.BassEngine):
            # absorb the exp table load while inputs stream in
            act.wait_ge(sWARM, 1)
            act.activation(warm, warm, EXP)

            # fused rounds: exp-A(r) then exp-B(r)
            for r in range(4 * IT):
                t, g = r // 4, r % 4
                u = r % 2
                act.wait_ge(sWMA, r + 1)
                if g == 0 and t >= 2:
                    act.wait_ge(sNRM, t - 1)
                act.activation(Et[:, t % 2, bass.ts(g, 1024)],
                               psA[u][:, 0:1024], EXP,
                               accum_out=sp[:, t, g : g + 1]
                               ).then_inc(sEXA)
                act.wait_ge(sWMB, r + 1)
                if r >= 3:
                    act.wait_ge(sOMM, r - 2)
                act.activation(ETb[:, r % 3, :], psB[:, 0:1024],
                               EXP).then_inc(sEXB)

        @block.vector
        def _(dve: bass.BassEngine):
            # phase A: rowsum -> reciprocal -> normalize
            for t in range(IT):
                dve.wait_ge(sEXA, 4 * t + 4)
                if t >= 3:
                    dve.wait_ge(sPOUTs[t - 3], 16)
                dve.tensor_reduce(ssum[:, t, :], sp[:, t, :],
                                  mybir.AxisListType.X, mybir.AluOpType.add
                                  ).then_inc(sDVE)
                dve.wait_ge(sDVE, 2 * t + 1)
                dve.reciprocal(srec[:, t, :], ssum[:, t, :]).then_inc(sDVE)
                dve.wait_ge(sDVE, 2 * t + 2)
                dve.tensor_scalar_mul(PT(t), Et[:, t % 2, :],
                                      srec[:, t, :]).then_inc(sNRM)
                if t == 7:
                    # drain pair-0 out accumulators so psO can be reused
                    dve.wait_ge(sOMM, JC)
                    dve.tensor_copy(out_raw[:, bass.ds(0, 512)], psO[0]
                                    ).then_inc(sOCP)
                    dve.tensor_copy(out_raw[:, bass.ds(512, 512)], psO[1]
                                    ).then_inc(sOCP)

            # drain pair-1 out accumulators
            dve.wait_ge(sOMM, 2 * JC)
            dve.tensor_copy(out_raw[:, bass.ds(1024, 512)], psO[0]
                            ).then_inc(sOCP)
            dve.tensor_copy(out_raw[:, bass.ds(1536, 512)], psO[1]
                            ).then_inc(sOCP)

    return nc


def _split_bf16(x32):
    hi = x32.astype(BF16)
    lo = (x32 - hi.astype(np.float32)).astype(BF16)
    return hi, lo


def kernel(feat, pred, seed, conv_w, conv_b, bn_gamma, bn_beta, bn_mean,
           bn_var, alpha, beta):
    global _CACHED_NC
    feat = np.asarray(feat, dtype=np.float32)
    pred = np.asarray(pred, dtype=np.float32)
    seed = np.asarray(seed, dtype=np.float32)
    conv_w = np.asarray(conv_w, dtype=np.float32)
    conv_b = np.asarray(conv_b, dtype=np.float32)
    bn_gamma = np.asarray(bn_gamma, dtype=np.float32)
    bn_beta = np.asarray(bn_beta, dtype=np.float32)
    bn_mean = np.asarray(bn_mean, dtype=np.float32)
    bn_var = np.asarray(bn_var, dtype=np.float32)
    a = np.float32(np.asarray(alpha).reshape(-1)[0])
    bt = np.float32(np.asarray(beta).reshape(-1)[0])

    # fold BN into the 1x1 conv
    scale = (bn_gamma / np.sqrt(bn_var + np.float32(BN_EPS))).astype(np.float32)
    shift = (bn_beta - bn_mean * scale).astype(np.float32)
    Wf = (conv_w * scale[:, None]).astype(np.float32)        # [64, 512]
    bf_ = (conv_b * scale + shift).astype(np.float32)        # [64]

    # fold alpha/beta sigmoids
    def sig(x):
        return np.float32(1.0) / (np.float32(1.0) + np.exp(-x, dtype=np.float32))

    sa_n, sa_p = sig(-a), sig(a)
    sb_n, sb_p = sig(-bt), sig(bt)
    k1 = np.float32(sb_n * sa_n)
    k2 = np.float32(sb_n * sa_p)
    k3 = np.float32(sb_p)

    if _CACHED_NC is None:
        _CACHED_NC = build_kernel()
    nc = _CACHED_NC

    # Each core computes rows [0:RB] of ITS view. Cores with r=1 see the
    # token axis rotated left by RB, so their "rows 0:RB" are the original
    # rows RB:2*RB (softmax is equivariant under token permutation).
    in_maps = []
    per_batch = {}
    for bi in range(B):
        fv = feat[bi].reshape(CIN, HW)
        # Fv = W' @ feat + b'  (exact fp32 on host; 0.5% of total FLOPs)
        Fv = (Wf @ fv + bf_[:, None]).astype(np.float32)     # [64, 4096]
        hi, lo = _split_bf16(Fv)
        ptk = np.ascontiguousarray(
            (k1 * pred[bi].reshape(C, HW)).T).astype(BF16)
        per_batch[bi] = (hi, lo, ptk)
    for core in range(NCORES):
        bi, r = core // 2, core % 2
        hi, lo, ptk = per_batch[bi]
        s = RB * r
        if s:
            hi = np.roll(hi, -s, axis=1)
            lo = np.roll(lo, -s, axis=1)
            ptk = np.ascontiguousarray(np.roll(ptk, -s, axis=0))
        fs = np.ascontiguousarray(np.concatenate([hi, lo], axis=0))
        fr = np.ascontiguousarray(np.concatenate([lo, hi], axis=0))
        fhh = np.ascontiguousarray(np.concatenate([hi, hi], axis=0))
        in_maps.append({"fs": fs, "fr": fr, "fhh": fhh, "ptk": ptk})

    res = run_bass_kernel_spmd(nc, in_maps, list(range(NCORES)))

    P = np.empty((B, HW, HW), dtype=np.float32)
    outv = np.empty((B, C, HW), dtype=np.float32)
    for core in range(NCORES):
        bi, r = core // 2, core % 2
        s = RB * r
        P_blk = res.results[core]["P_blk"]
        if s:
            P_blk = np.roll(P_blk, s, axis=1)
        P[bi, s : s + RB, :] = P_blk
        rs_vec = res.results[core]["rs_blk"].T.reshape(RB)
        outv[bi][:, s : s + RB] = res.results[core]["out_blk"] * rs_vec[None, :]

    out = (outv + k2 * seed.reshape(B, C, HW) + k3 * pred.reshape(B, C, HW))
    out = out.reshape(B, C, H, W).astype(np.float32)
    return out, P
